# revision 10
# baseline (speedup 1.0000x reference)
"""Self-contained Trainium2 Bass kernel for nn_MiMoMoeAttention.

Tensor-parallel over heads across 8 NeuronCores: each core owns 4 query
heads + 1 kv head (one GQA group); q/k/v projections column-sharded,
o_proj row-sharded, partial outputs summed on the host.

All matmuls run as fp32r (fp32 with mantissa RNE-rounded to 11 bits,
single-pass full-rate on the PE). Host pre-rounds weights/activations to
the fp32r encoding and pre-tiles them for contiguous 256KB DMAs.
"""
import numpy as np
from contextlib import ExitStack

from concourse import bacc
import concourse.tile as tile
import concourse.mybir as mybir
from concourse.alu_op_type import AluOpType
from concourse.bass_utils import run_bass_kernel_spmd

dt = mybir.dt
AF = mybir.ActivationFunctionType

B, S, HID = 1, 2048, 4096
H, HK, D = 32, 8, 128
WIN = 1024
THETA = 1000000.0
NCORES = 8
HQ = H // NCORES            # 4 query heads per core
CH = 512                    # token chunk width
NCH = S // CH               # 4 chunks
KT = HID // 128             # 32 contraction tiles
NE = HID // CH              # 8 o_proj column chunks
CBLK = CH // 128            # 4 query blocks per chunk
WBLK = WIN // 128           # 8 blocks lookback
SCALE = float(D) ** -0.5
MASK_NEG = -30000.0


def _round_f32r(x: np.ndarray) -> np.ndarray:
    """RNE-round fp32 to 11 explicit mantissa bits (the fp32r encoding)."""
    u = np.ascontiguousarray(x, dtype=np.float32).view(np.uint32)
    r = (u.astype(np.uint64) + 0x7FF + ((u >> 12) & 1)) & 0xFFFFF000
    return r.astype(np.uint32).view(np.float32)


def _build():
    nc = bacc.Bacc("TRN2", target_bir_lowering=False, debug=False,
                   num_devices=NCORES)
    f32, f32r = dt.float32, dt.float32r
    # hsT tiled host-side: row block (kt*NCH + c) holds hsT[kt, :, chunk c]
    hsT = nc.dram_tensor("hsT", [KT * NCH * 128, CH], f32r,
                         kind="ExternalInput").ap()
    # weights pre-tiled so SBUF destination is a contiguous 2D copy
    wq = nc.dram_tensor("wq", [128, KT * HQ * D], f32r,
                        kind="ExternalInput").ap()
    wk = nc.dram_tensor("wk", [128, KT * D], f32r, kind="ExternalInput").ap()
    wv = nc.dram_tensor("wv", [128, KT * D], f32r, kind="ExternalInput").ap()
    # wo tiled: row block (jt*NE + e) holds Wo[jt*128:(jt+1)*128, e*CH:+CH]
    wo = nc.dram_tensor("wo", [HQ * NE * 128, CH], f32r,
                        kind="ExternalInput").ap()
    bq = nc.dram_tensor("bq", [128, HQ], f32, kind="ExternalInput").ap()
    bk = nc.dram_tensor("bk", [128, 1], f32, kind="ExternalInput").ap()
    bv = nc.dram_tensor("bv", [128, 1], f32, kind="ExternalInput").ap()
    cosd = nc.dram_tensor("cosd", [128, S], f32, kind="ExternalInput").ap()
    sind = nc.dram_tensor("sind", [128, S], f32, kind="ExternalInput").ap()
    m0 = nc.dram_tensor("m0", [128, 128], f32, kind="ExternalInput").ap()
    m8 = nc.dram_tensor("m8", [128, 128], f32, kind="ExternalInput").ap()
    esink = nc.dram_tensor("esink", [1, HQ], f32, kind="ExternalInput").ap()
    ident = nc.dram_tensor("ident", [128, 128], f32, kind="ExternalInput").ap()
    pswap = nc.dram_tensor("pswap", [128, 128], f32r, kind="ExternalInput").ap()
    onc = nc.dram_tensor("onc", [128, 1], f32r, kind="ExternalInput").ap()
    onr = nc.dram_tensor("onr", [1, 128], f32r, kind="ExternalInput").ap()
    out = nc.dram_tensor("o_part", [S, HID], f32, kind="ExternalOutput").ap()

    with tile.TileContext(nc) as tc, ExitStack() as ctx:
        const = ctx.enter_context(tc.tile_pool(name="const", bufs=1))
        keep = ctx.enter_context(tc.tile_pool(name="keep", bufs=1))
        work = ctx.enter_context(tc.tile_pool(name="work", bufs=1))
        ps = ctx.enter_context(tc.tile_pool(name="ps", bufs=1, space="PSUM"))

        # ---- preload constants / weights -------------------------------
        wq_sb = const.tile([128, KT * HQ * D], f32r, tag="wq", name="wq_sb")
        nc.sync.dma_start(wq_sb[:], wq)
        wk_sb = const.tile([128, KT * D], f32r, tag="wk", name="wk_sb")
        nc.sync.dma_start(wk_sb[:], wk)
        wv_sb = const.tile([128, KT * D], f32r, tag="wv", name="wv_sb")
        nc.sync.dma_start(wv_sb[:], wv)
        bq_sb = const.tile([128, HQ], f32, tag="bq", name="bq_sb")
        nc.sync.dma_start(bq_sb[:], bq)
        bk_sb = const.tile([128, 1], f32, tag="bk", name="bk_sb")
        nc.sync.dma_start(bk_sb[:], bk)
        bv_sb = const.tile([128, 1], f32, tag="bv", name="bv_sb")
        nc.sync.dma_start(bv_sb[:], bv)
        m0_sb = const.tile([128, 128], f32, tag="m0", name="m0_sb")
        nc.sync.dma_start(m0_sb[:], m0)
        m8_sb = const.tile([128, 128], f32, tag="m8", name="m8_sb")
        nc.sync.dma_start(m8_sb[:], m8)
        es_sb = const.tile([1, HQ], f32, tag="es", name="es_sb")
        nc.sync.dma_start(es_sb[:], esink)
        id_sb = const.tile([128, 128], f32, tag="ident", name="id_sb")
        nc.sync.dma_start(id_sb[:], ident)
        pw_sb = const.tile([128, 128], f32r, tag="pswap", name="pw_sb")
        nc.sync.dma_start(pw_sb[:], pswap)
        ones_c = const.tile([128, 1], f32r, tag="ones_c", name="ones_c")
        nc.sync.dma_start(ones_c[:], onc)
        ones_r = const.tile([1, 128], f32r, tag="ones_r", name="ones_r")
        nc.sync.dma_start(ones_r[:], onr)

        # persistent rotated K (d-major) and V (t-major) for all tokens
        krotT = keep.tile([128, S], f32r, tag="krotT", name="krotT")
        v_all = keep.tile([128, S], f32r, tag="v_all", name="v_all")

        def rope(dst, src_sb, swap_ps, cos_sb, sin_sb):
            """dst = src*cos + swap*sinS, full 128-partition ops.
            sinS has rows 0:64 = -sin (so top half gets x1c - x2s) and
            rows 64:128 = +sin (bottom half gets x2c + x1s)."""
            t1 = work.tile([128, CH], f32, tag="r1", bufs=2, name="t1")
            nc.vector.tensor_tensor(t1[:], src_sb[:], cos_sb[:],
                                    op=AluOpType.mult)
            t2 = work.tile([128, CH], f32, tag="r2", bufs=2, name="t2")
            nc.vector.tensor_tensor(t2[:], swap_ps[:], sin_sb[:],
                                    op=AluOpType.mult)
            nc.vector.tensor_tensor(dst, t1[:], t2[:], op=AluOpType.add)

        def attn_pass(heads, qrot, blo, bhi, s0, an_out):
            """Sliding-window attention for a pair of heads, j-outer,
            software-pipelined so PE never waits on the exp."""
            at = {}
            dn = {}
            for idx, h in enumerate(heads):
                at[h] = ps.tile([128, CH], f32, tag=f"a{idx}", name="at_ps")
                nc.vector.memset(at[h][:], 0.0)
                dn[h] = ps.tile([1, CH], f32, tag=f"a{idx + 2}", name="dn_ps")
                nc.vector.memset(dn[h][:], 0.0)
            lgi = 0
            pend = []

            def consume(item):
                h, E, c0, w, j = item
                nc.tensor.matmul(dn[h][:, c0:c0 + w], ones_c[:], E[:, :w],
                                 start=False, stop=True)
                nc.tensor.matmul(at[h][:, c0:c0 + w],
                                 v_all[:, j * 128:(j + 1) * 128],
                                 E[:, :w], start=False, stop=True)

            for j in range(max(0, blo - WBLK), bhi + 1):
                lo, hi = max(j, blo), min(j + WBLK, bhi)
                c0 = (lo - blo) * 128
                w = (hi - lo + 1) * 128
                has_m0 = lo == j
                has_m8 = hi == j + WBLK
                for h in heads:
                    lg = ps.tile([128, CH], f32,
                                 tag=("c0", "c1", "b0", "b1")[lgi % 4],
                                 name="lg")
                    lgi += 1
                    nc.tensor.matmul(lg[:, :w],
                                     krotT[:, j * 128:(j + 1) * 128],
                                     qrot[h][:, c0:c0 + w],
                                     start=True, stop=True)
                    E = work.tile([128, CH], f32r, tag="E", bufs=6, name="E")
                    nc.scalar.activation(E[:, :w], lg[:, :w], AF.Exp,
                                         scale=SCALE)
                    if has_m0:
                        nc.vector.tensor_tensor(E[:, 0:128], E[:, 0:128],
                                                m0_sb[:], op=AluOpType.mult)
                    if has_m8:
                        nc.vector.tensor_tensor(E[:, w - 128:w],
                                                E[:, w - 128:w],
                                                m8_sb[:], op=AluOpType.mult)
                    pend.append((h, E, c0, w, j))
                while len(pend) > 4:
                    consume(pend.pop(0))
            for item in pend:
                consume(item)
            for idx, h in enumerate(heads):
                lnd = work.tile([1, CH], f32, tag="lnd", bufs=2, name="lnd")
                nc.scalar.activation(lnd[:], dn[h][:], AF.Ln,
                                     bias=es_sb[0:1, h:h + 1])
                rcp = work.tile([1, CH], f32r, tag="rcp", bufs=2, name="rcp")
                nc.scalar.activation(rcp[:], lnd[:], AF.Exp, scale=-1.0)
                rb_ps = ps.tile([128, CH], f32, tag=f"b{idx}", name="rb_ps")
                nc.tensor.matmul(rb_ps[:], ones_r[:], rcp[:],
                                 start=True, stop=True)
                rb_sb = work.tile([128, CH], f32, tag="rb", bufs=2,
                                  name="rb_sb")
                nc.vector.tensor_copy(rb_sb[:], rb_ps[:])
                an = work.tile([128, CH], f32r, tag="an", bufs=4, name="an")
                nc.vector.tensor_tensor(an[:], at[h][:], rb_sb[:],
                                        op=AluOpType.mult)
                an_out[h] = an

        def qprep(jt, q_ps, cos_sb, sin_sb):
            qT_sb = work.tile([128, CH], f32r, tag="pt", bufs=3, name="qT_sb")
            nc.vector.tensor_scalar_add(qT_sb[:], q_ps[:], bq_sb[:, jt:jt + 1])
            qsw = ps.tile([128, CH], f32, tag=f"a{jt}", name="qsw")
            nc.tensor.matmul(qsw[:], pw_sb[:], qT_sb[:], start=True, stop=True)
            qr = work.tile([128, CH], f32r, tag="qrot", bufs=4, name="qr")
            rope(qr[:], qT_sb[:], qsw[:], cos_sb[:], sin_sb[:])
            return qr

        for c in range(NCH):
            s0 = c * CH
            cos_sb = work.tile([128, CH], f32, tag="cos", bufs=2, name="cos_sb")
            nc.sync.dma_start(cos_sb[:], cosd[:, s0:s0 + CH])
            sin_sb = work.tile([128, CH], f32, tag="sin", bufs=2, name="sin_sb")
            nc.sync.dma_start(sin_sb[:], sind[:, s0:s0 + CH])

            # ---- fused q/k/v projection for this token chunk ----------
            q_ps = [ps.tile([128, CH], f32, tag=f"a{jt}", name=f"q_ps{jt}")
                    for jt in range(HQ)]
            k_ps = ps.tile([128, CH], f32, tag="b0", name="k_ps")
            v_ps = ps.tile([128, CH], f32, tag="b1", name="v_ps")
            for kt in range(KT):
                hst = work.tile([128, CH], f32r, tag="hst", bufs=6, name="hst")
                nc.sync.dma_start(
                    hst[:], hsT[(kt * NCH + c) * 128:(kt * NCH + c + 1) * 128])
                first, last = kt == 0, kt == KT - 1
                for jt in range(HQ):
                    nc.tensor.matmul(
                        q_ps[jt][:],
                        wq_sb[:, kt * HQ * D + jt * D:kt * HQ * D + (jt + 1) * D],
                        hst[:], start=first, stop=last)
                nc.tensor.matmul(k_ps[:], wk_sb[:, kt * D:(kt + 1) * D],
                                 hst[:], start=first, stop=last)
                nc.tensor.matmul(v_ps[:], wv_sb[:, kt * D:(kt + 1) * D],
                                 hst[:], start=first, stop=last)

            # ---- K: bias, half-swap (PE), rope -> krotT[:, chunk] -----
            kT_sb = work.tile([128, CH], f32r, tag="pt", bufs=3, name="kT_sb")
            nc.vector.tensor_scalar_add(kT_sb[:], k_ps[:], bk_sb[:])
            ksw = ps.tile([128, CH], f32, tag="b0", name="ksw")
            nc.tensor.matmul(ksw[:], pw_sb[:], kT_sb[:], start=True, stop=True)
            rope(krotT[:, s0:s0 + CH], kT_sb[:], ksw[:], cos_sb[:], sin_sb[:])

            # ---- V: bias, transpose to t-major -> v_all[:, chunk] -----
            vT_sb = work.tile([128, CH], f32, tag="pt", bufs=3, name="vT_sb")
            nc.vector.tensor_scalar_add(vT_sb[:], v_ps[:], bv_sb[:])
            for i in range(CBLK):
                vt = ps.tile([128, 128], f32, tag="b1", name="vt")
                nc.tensor.transpose(vt[:], vT_sb[:, i * 128:(i + 1) * 128],
                                    id_sb[:])
                nc.scalar.copy(
                    v_all[:, s0 + i * 128:s0 + (i + 1) * 128], vt[:])

            # ---- Q prep + attention in two head-pair passes -----------
            blo, bhi = c * CBLK, c * CBLK + CBLK - 1
            qrot = {}
            an_out = {}
            qrot[0] = qprep(0, q_ps[0], cos_sb, sin_sb)
            qrot[1] = qprep(1, q_ps[1], cos_sb, sin_sb)
            attn_pass((0, 1), qrot, blo, bhi, s0, an_out)
            qrot[2] = qprep(2, q_ps[2], cos_sb, sin_sb)
            qrot[3] = qprep(3, q_ps[3], cos_sb, sin_sb)
            attn_pass((2, 3), qrot, blo, bhi, s0, an_out)

            # ---- o_proj (row shard): out[s, e] += attn.T @ Wo ---------
            for e in range(NE):
                wo_t = []
                for jt in range(HQ):
                    wt = work.tile([128, CH], f32r, tag="wo", bufs=4,
                                   name="wo_t")
                    nc.sync.dma_start(
                        wt[:],
                        wo[(jt * NE + e) * 128:(jt * NE + e + 1) * 128])
                    wo_t.append(wt)
                for sb in range(CBLK):
                    o_ps = ps.tile([128, CH], f32, tag=f"a{sb % 2}",
                                   name="o_ps")
                    for jt in range(HQ):
                        nc.tensor.matmul(
                            o_ps[:], an_out[jt][:, sb * 128:(sb + 1) * 128],
                            wo_t[jt][:], start=jt == 0, stop=jt == HQ - 1)
                    o_sb = work.tile([128, CH], f32, tag="osb", bufs=4,
                                     name="o_sb")
                    if (e + sb) % 2 == 0:
                        nc.scalar.copy(o_sb[:], o_ps[:])
                    else:
                        nc.vector.tensor_copy(o_sb[:], o_ps[:])
                    nc.sync.dma_start(
                        out[s0 + sb * 128:s0 + (sb + 1) * 128,
                            e * CH:(e + 1) * CH], o_sb[:])

    nc.compile()
    return nc


_CACHED = None
_LAST_IN_MAPS = None


def _get_nc():
    global _CACHED
    if _CACHED is None:
        _CACHED = _build()
    return _CACHED


def kernel(positions, hidden_states, Wq, bq, Wk, bk, Wv, bv, Wo, sink,
           **_ignored):
    positions = np.asarray(positions)
    hidden_states = np.asarray(hidden_states, dtype=np.float32)
    Wq = np.asarray(Wq, dtype=np.float32)
    Wk = np.asarray(Wk, dtype=np.float32)
    Wv = np.asarray(Wv, dtype=np.float32)
    Wo = np.asarray(Wo, dtype=np.float32)
    bq = np.asarray(bq, dtype=np.float32)
    bk = np.asarray(bk, dtype=np.float32)
    bv = np.asarray(bv, dtype=np.float32)
    sink = np.asarray(sink, dtype=np.float32)

    # host-derived tables
    half = D // 2
    inv_freq = 1.0 / (THETA ** (np.arange(half, dtype=np.float64) / half))
    ang = positions[0].astype(np.float64)[None, :] * inv_freq[:, None]  # [64,S]
    cos64 = np.cos(ang).astype(np.float32)
    sin64 = np.sin(ang).astype(np.float32)
    cosd = np.ascontiguousarray(np.concatenate([cos64, cos64], axis=0))
    # signed sin: top half -sin (x1c - x2s), bottom half +sin (x2c + x1s)
    sind = np.ascontiguousarray(np.concatenate([-sin64, sin64], axis=0))
    r, cidx = np.arange(128)[:, None], np.arange(128)[None, :]
    m0 = (r <= cidx).astype(np.float32)
    m8 = (r > cidx).astype(np.float32)
    ident = np.eye(128, dtype=np.float32)
    pswap = np.zeros((128, 128), dtype=np.float32)
    pswap[np.arange(128), (np.arange(128) + 64) % 128] = 1.0

    # hsT tiled: [KT*NCH*128, CH]; row block (kt*NCH + c) = hsT tile
    hsT_full = _round_f32r(np.ascontiguousarray(hidden_states[0].T))
    hsT_t = np.ascontiguousarray(
        hsT_full.reshape(KT, 128, NCH, CH).transpose(0, 2, 1, 3)
        .reshape(KT * NCH * 128, CH))
    esink_all = np.exp(sink.astype(np.float64)).astype(np.float32)

    in_maps = []
    for core in range(NCORES):
        qs = slice(core * HQ * D, (core + 1) * HQ * D)
        ks = slice(core * D, (core + 1) * D)
        # weights pre-tiled: [128, KT*cols] with kt blocks side by side
        wq_c = _round_f32r(Wq[:, qs]).reshape(KT, 128, HQ * D)
        wq_t = np.ascontiguousarray(
            wq_c.transpose(1, 0, 2).reshape(128, KT * HQ * D))
        wk_t = np.ascontiguousarray(
            _round_f32r(Wk[:, ks]).reshape(KT, 128, D)
            .transpose(1, 0, 2).reshape(128, KT * D))
        wv_t = np.ascontiguousarray(
            _round_f32r(Wv[:, ks]).reshape(KT, 128, D)
            .transpose(1, 0, 2).reshape(128, KT * D))
        # wo tiled: [HQ*NE*128, CH]
        wo_t = np.ascontiguousarray(
            _round_f32r(Wo[qs, :]).reshape(HQ, 128, NE, CH)
            .transpose(0, 2, 1, 3).reshape(HQ * NE * 128, CH))
        in_maps.append(dict(
            hsT=hsT_t, wq=wq_t, wk=wk_t, wv=wv_t, wo=wo_t,
            bq=np.ascontiguousarray(bq[qs].reshape(HQ, D).T),
            bk=np.ascontiguousarray(bk[ks].reshape(D, 1)),
            bv=np.ascontiguousarray(bv[ks].reshape(D, 1)),
            cosd=cosd, sind=sind, m0=m0, m8=m8,
            esink=np.ascontiguousarray(
                esink_all[core * HQ:(core + 1) * HQ].reshape(1, HQ)),
            ident=ident, pswap=pswap,
            onc=np.ones((128, 1), dtype=np.float32),
            onr=np.ones((1, 128), dtype=np.float32),
        ))

    global _LAST_IN_MAPS
    _LAST_IN_MAPS = in_maps
    nc = _get_nc()
    res = run_bass_kernel_spmd(nc, in_maps, list(range(NCORES)))
    out = np.zeros((S, HID), dtype=np.float64)
    for core in range(NCORES):
        out += res.results[core]["o_part"].astype(np.float64)
    return out.astype(np.float32).reshape(B, S, HID)


# revision 11
# speedup vs baseline: 1.0403x; 1.0403x over previous
"""Self-contained Trainium2 Bass kernel for nn_MiMoMoeAttention.

Tensor-parallel over heads across 8 NeuronCores: each core owns 4 query
heads + 1 kv head (one GQA group); q/k/v projections column-sharded,
o_proj row-sharded, partial outputs summed on the host.

All matmuls run as fp32r (fp32 with mantissa RNE-rounded to 11 bits,
single-pass full-rate on the PE). Host pre-rounds weights/activations to
the fp32r encoding and pre-tiles them for contiguous 256KB DMAs.
"""
import numpy as np
from contextlib import ExitStack

from concourse import bacc
import concourse.tile as tile
import concourse.mybir as mybir
from concourse.alu_op_type import AluOpType
from concourse.bass_utils import run_bass_kernel_spmd

dt = mybir.dt
AF = mybir.ActivationFunctionType

B, S, HID = 1, 2048, 4096
H, HK, D = 32, 8, 128
WIN = 1024
THETA = 1000000.0
NCORES = 8
HQ = H // NCORES            # 4 query heads per core
CH = 512                    # token chunk width
NCH = S // CH               # 4 chunks
KT = HID // 128             # 32 contraction tiles
NE = HID // CH              # 8 o_proj column chunks
CBLK = CH // 128            # 4 query blocks per chunk
WBLK = WIN // 128           # 8 blocks lookback
SCALE = float(D) ** -0.5
MASK_NEG = -30000.0


def _round_f32r(x: np.ndarray) -> np.ndarray:
    """RNE-round fp32 to 11 explicit mantissa bits (the fp32r encoding)."""
    u = np.ascontiguousarray(x, dtype=np.float32).view(np.uint32)
    r = (u.astype(np.uint64) + 0x7FF + ((u >> 12) & 1)) & 0xFFFFF000
    return r.astype(np.uint32).view(np.float32)


def _build():
    nc = bacc.Bacc("TRN2", target_bir_lowering=False, debug=False,
                   num_devices=NCORES)
    f32, f32r = dt.float32, dt.float32r
    # hsT tiled host-side: row block (kt*NCH + c) holds hsT[kt, :, chunk c]
    hsT = nc.dram_tensor("hsT", [KT * NCH * 128, CH], f32r,
                         kind="ExternalInput").ap()
    # weights pre-tiled so SBUF destination is a contiguous 2D copy
    wq = nc.dram_tensor("wq", [128, KT * HQ * D], f32r,
                        kind="ExternalInput").ap()
    wk = nc.dram_tensor("wk", [128, KT * D], f32r, kind="ExternalInput").ap()
    wv = nc.dram_tensor("wv", [128, KT * D], f32r, kind="ExternalInput").ap()
    # wo tiled: row block (jt*NE + e) holds Wo[jt*128:(jt+1)*128, e*CH:+CH]
    wo = nc.dram_tensor("wo", [HQ * NE * 128, CH], f32r,
                        kind="ExternalInput").ap()
    bq = nc.dram_tensor("bq", [128, HQ], f32, kind="ExternalInput").ap()
    bk = nc.dram_tensor("bk", [128, 1], f32, kind="ExternalInput").ap()
    bv = nc.dram_tensor("bv", [128, 1], f32, kind="ExternalInput").ap()
    cosd = nc.dram_tensor("cosd", [128, S], f32, kind="ExternalInput").ap()
    sind = nc.dram_tensor("sind", [128, S], f32, kind="ExternalInput").ap()
    m0 = nc.dram_tensor("m0", [128, 128], f32, kind="ExternalInput").ap()
    m8 = nc.dram_tensor("m8", [128, 128], f32, kind="ExternalInput").ap()
    esink = nc.dram_tensor("esink", [1, HQ], f32, kind="ExternalInput").ap()
    ident = nc.dram_tensor("ident", [128, 128], f32, kind="ExternalInput").ap()
    pswap = nc.dram_tensor("pswap", [128, 128], f32r, kind="ExternalInput").ap()
    onc = nc.dram_tensor("onc", [128, 1], f32r, kind="ExternalInput").ap()
    onr = nc.dram_tensor("onr", [1, 128], f32r, kind="ExternalInput").ap()
    out = nc.dram_tensor("o_part", [S, HID], f32, kind="ExternalOutput").ap()

    with tile.TileContext(nc) as tc, ExitStack() as ctx:
        const = ctx.enter_context(tc.tile_pool(name="const", bufs=1))
        keep = ctx.enter_context(tc.tile_pool(name="keep", bufs=1))
        work = ctx.enter_context(tc.tile_pool(name="work", bufs=1))
        ps = ctx.enter_context(tc.tile_pool(name="ps", bufs=1, space="PSUM"))

        # ---- preload constants / weights -------------------------------
        wq_sb = const.tile([128, KT * HQ * D], f32r, tag="wq", name="wq_sb")
        nc.sync.dma_start(wq_sb[:], wq)
        wk_sb = const.tile([128, KT * D], f32r, tag="wk", name="wk_sb")
        nc.sync.dma_start(wk_sb[:], wk)
        wv_sb = const.tile([128, KT * D], f32r, tag="wv", name="wv_sb")
        nc.sync.dma_start(wv_sb[:], wv)
        bq_sb = const.tile([128, HQ], f32, tag="bq", name="bq_sb")
        nc.sync.dma_start(bq_sb[:], bq)
        bk_sb = const.tile([128, 1], f32, tag="bk", name="bk_sb")
        nc.sync.dma_start(bk_sb[:], bk)
        bv_sb = const.tile([128, 1], f32, tag="bv", name="bv_sb")
        nc.sync.dma_start(bv_sb[:], bv)
        m0_sb = const.tile([128, 128], f32, tag="m0", name="m0_sb")
        nc.sync.dma_start(m0_sb[:], m0)
        m8_sb = const.tile([128, 128], f32, tag="m8", name="m8_sb")
        nc.sync.dma_start(m8_sb[:], m8)
        es_sb = const.tile([1, HQ], f32, tag="es", name="es_sb")
        nc.sync.dma_start(es_sb[:], esink)
        id_sb = const.tile([128, 128], f32, tag="ident", name="id_sb")
        nc.sync.dma_start(id_sb[:], ident)
        pw_sb = const.tile([128, 128], f32r, tag="pswap", name="pw_sb")
        nc.sync.dma_start(pw_sb[:], pswap)
        ones_c = const.tile([128, 1], f32r, tag="ones_c", name="ones_c")
        nc.sync.dma_start(ones_c[:], onc)
        ones_r = const.tile([1, 128], f32r, tag="ones_r", name="ones_r")
        nc.sync.dma_start(ones_r[:], onr)

        # persistent rotated K (d-major) and V (t-major) for all tokens
        krotT = keep.tile([128, S], f32r, tag="krotT", name="krotT")
        v_all = keep.tile([128, S], f32r, tag="v_all", name="v_all")

        def rope(dst, src_sb, swap_ps, cos_sb, sin_sb):
            """dst = src*cos + swap*sinS, full 128-partition ops.
            sinS has rows 0:64 = -sin (so top half gets x1c - x2s) and
            rows 64:128 = +sin (bottom half gets x2c + x1s)."""
            t1 = work.tile([128, CH], f32, tag="r1", bufs=2, name="t1")
            nc.vector.tensor_tensor(t1[:], src_sb[:], cos_sb[:],
                                    op=AluOpType.mult)
            t2 = work.tile([128, CH], f32, tag="r2", bufs=2, name="t2")
            nc.vector.tensor_tensor(t2[:], swap_ps[:], sin_sb[:],
                                    op=AluOpType.mult)
            nc.vector.tensor_tensor(dst, t1[:], t2[:], op=AluOpType.add)

        def attn_pass(heads, qrot, blo, bhi, s0, an_out):
            """Sliding-window attention for a pair of heads, j-outer,
            software-pipelined so PE never waits on the exp."""
            at = {}
            dn = {}
            for idx, h in enumerate(heads):
                at[h] = ps.tile([128, CH], f32, tag=f"a{idx}", name="at_ps")
                nc.vector.memset(at[h][:], 0.0)
                dn[h] = ps.tile([1, CH], f32, tag=f"a{idx + 2}", name="dn_ps")
                nc.vector.memset(dn[h][:], 0.0)
            lgi = 0
            pend = []

            def consume(item):
                h, E, c0, w, j = item
                nc.tensor.matmul(dn[h][:, c0:c0 + w], ones_c[:], E[:, :w],
                                 start=False, stop=True)
                nc.tensor.matmul(at[h][:, c0:c0 + w],
                                 v_all[:, j * 128:(j + 1) * 128],
                                 E[:, :w], start=False, stop=True)

            for j in range(max(0, blo - WBLK), bhi + 1):
                lo, hi = max(j, blo), min(j + WBLK, bhi)
                c0 = (lo - blo) * 128
                w = (hi - lo + 1) * 128
                has_m0 = lo == j
                has_m8 = hi == j + WBLK
                for h in heads:
                    lg = ps.tile([128, CH], f32,
                                 tag=("c0", "c1", "b0", "b1")[lgi % 4],
                                 name="lg")
                    lgi += 1
                    nc.tensor.matmul(lg[:, :w],
                                     krotT[:, j * 128:(j + 1) * 128],
                                     qrot[h][:, c0:c0 + w],
                                     start=True, stop=True)
                    E = work.tile([128, CH], f32r, tag="E", bufs=6, name="E")
                    nc.scalar.activation(E[:, :w], lg[:, :w], AF.Exp,
                                         scale=SCALE)
                    if has_m0:
                        nc.vector.tensor_tensor(E[:, 0:128], E[:, 0:128],
                                                m0_sb[:], op=AluOpType.mult)
                    if has_m8:
                        nc.vector.tensor_tensor(E[:, w - 128:w],
                                                E[:, w - 128:w],
                                                m8_sb[:], op=AluOpType.mult)
                    pend.append((h, E, c0, w, j))
                while len(pend) > 4:
                    consume(pend.pop(0))
            for item in pend:
                consume(item)
            for idx, h in enumerate(heads):
                lnd = work.tile([1, CH], f32, tag="lnd", bufs=2, name="lnd")
                nc.scalar.activation(lnd[:], dn[h][:], AF.Ln,
                                     bias=es_sb[0:1, h:h + 1])
                rcp = work.tile([1, CH], f32r, tag="rcp", bufs=2, name="rcp")
                nc.scalar.activation(rcp[:], lnd[:], AF.Exp, scale=-1.0)
                rb_ps = ps.tile([128, CH], f32, tag=f"b{idx}", name="rb_ps")
                nc.tensor.matmul(rb_ps[:], ones_r[:], rcp[:],
                                 start=True, stop=True)
                rb_sb = work.tile([128, CH], f32, tag="rb", bufs=2,
                                  name="rb_sb")
                nc.scalar.copy(rb_sb[:], rb_ps[:])
                an = work.tile([128, CH], f32r, tag="an", bufs=4, name="an")
                nc.vector.tensor_tensor(an[:], at[h][:], rb_sb[:],
                                        op=AluOpType.mult)
                an_out[h] = an

        def qprep(jt, q_ps, cos_sb, sin_sb):
            qT_sb = work.tile([128, CH], f32r, tag="pt", bufs=3, name="qT_sb")
            nc.scalar.activation(qT_sb[:], q_ps[:], AF.Identity,
                                 bias=bq_sb[:, jt:jt + 1])
            qsw = ps.tile([128, CH], f32, tag=f"a{jt}", name="qsw")
            nc.tensor.matmul(qsw[:], pw_sb[:], qT_sb[:], start=True, stop=True)
            qr = work.tile([128, CH], f32r, tag="qrot", bufs=4, name="qr")
            rope(qr[:], qT_sb[:], qsw[:], cos_sb[:], sin_sb[:])
            return qr

        for c in range(NCH):
            s0 = c * CH
            cos_sb = work.tile([128, CH], f32, tag="cos", bufs=2, name="cos_sb")
            nc.sync.dma_start(cos_sb[:], cosd[:, s0:s0 + CH])
            sin_sb = work.tile([128, CH], f32, tag="sin", bufs=2, name="sin_sb")
            nc.sync.dma_start(sin_sb[:], sind[:, s0:s0 + CH])

            # ---- fused q/k/v projection for this token chunk ----------
            q_ps = [ps.tile([128, CH], f32, tag=f"a{jt}", name=f"q_ps{jt}")
                    for jt in range(HQ)]
            k_ps = ps.tile([128, CH], f32, tag="b0", name="k_ps")
            v_ps = ps.tile([128, CH], f32, tag="b1", name="v_ps")
            for kt in range(KT):
                hst = work.tile([128, CH], f32r, tag="hst", bufs=6, name="hst")
                nc.sync.dma_start(
                    hst[:], hsT[(kt * NCH + c) * 128:(kt * NCH + c + 1) * 128])
                first, last = kt == 0, kt == KT - 1
                for jt in range(HQ):
                    nc.tensor.matmul(
                        q_ps[jt][:],
                        wq_sb[:, kt * HQ * D + jt * D:kt * HQ * D + (jt + 1) * D],
                        hst[:], start=first, stop=last)
                nc.tensor.matmul(k_ps[:], wk_sb[:, kt * D:(kt + 1) * D],
                                 hst[:], start=first, stop=last)
                nc.tensor.matmul(v_ps[:], wv_sb[:, kt * D:(kt + 1) * D],
                                 hst[:], start=first, stop=last)

            # ---- K: bias, half-swap (PE), rope -> krotT[:, chunk] -----
            kT_sb = work.tile([128, CH], f32r, tag="pt", bufs=3, name="kT_sb")
            nc.scalar.activation(kT_sb[:], k_ps[:], AF.Identity, bias=bk_sb[:])
            ksw = ps.tile([128, CH], f32, tag="b0", name="ksw")
            nc.tensor.matmul(ksw[:], pw_sb[:], kT_sb[:], start=True, stop=True)
            rope(krotT[:, s0:s0 + CH], kT_sb[:], ksw[:], cos_sb[:], sin_sb[:])

            # ---- V: bias, transpose to t-major -> v_all[:, chunk] -----
            vT_sb = work.tile([128, CH], f32, tag="pt", bufs=3, name="vT_sb")
            nc.scalar.activation(vT_sb[:], v_ps[:], AF.Identity, bias=bv_sb[:])
            for i in range(CBLK):
                vt = ps.tile([128, 128], f32, tag="b1", name="vt")
                nc.tensor.transpose(vt[:], vT_sb[:, i * 128:(i + 1) * 128],
                                    id_sb[:])
                nc.scalar.copy(
                    v_all[:, s0 + i * 128:s0 + (i + 1) * 128], vt[:])

            # ---- Q prep + attention in two head-pair passes -----------
            blo, bhi = c * CBLK, c * CBLK + CBLK - 1
            qrot = {}
            an_out = {}
            qrot[0] = qprep(0, q_ps[0], cos_sb, sin_sb)
            qrot[1] = qprep(1, q_ps[1], cos_sb, sin_sb)
            attn_pass((0, 1), qrot, blo, bhi, s0, an_out)
            qrot[2] = qprep(2, q_ps[2], cos_sb, sin_sb)
            qrot[3] = qprep(3, q_ps[3], cos_sb, sin_sb)
            attn_pass((2, 3), qrot, blo, bhi, s0, an_out)

            # ---- o_proj (row shard): out[s, e] += attn.T @ Wo ---------
            for e in range(NE):
                wo_t = []
                for jt in range(HQ):
                    wt = work.tile([128, CH], f32r, tag="wo", bufs=4,
                                   name="wo_t")
                    nc.sync.dma_start(
                        wt[:],
                        wo[(jt * NE + e) * 128:(jt * NE + e + 1) * 128])
                    wo_t.append(wt)
                for sb in range(CBLK):
                    o_ps = ps.tile([128, CH], f32, tag=f"a{sb % 2}",
                                   name="o_ps")
                    for jt in range(HQ):
                        nc.tensor.matmul(
                            o_ps[:], an_out[jt][:, sb * 128:(sb + 1) * 128],
                            wo_t[jt][:], start=jt == 0, stop=jt == HQ - 1)
                    o_sb = work.tile([128, CH], f32, tag="osb", bufs=4,
                                     name="o_sb")
                    if (e + sb) % 2 == 0:
                        nc.scalar.copy(o_sb[:], o_ps[:])
                    else:
                        nc.vector.tensor_copy(o_sb[:], o_ps[:])
                    nc.sync.dma_start(
                        out[s0 + sb * 128:s0 + (sb + 1) * 128,
                            e * CH:(e + 1) * CH], o_sb[:])

    nc.compile()
    return nc


_CACHED = None
_LAST_IN_MAPS = None


def _get_nc():
    global _CACHED
    if _CACHED is None:
        _CACHED = _build()
    return _CACHED


def kernel(positions, hidden_states, Wq, bq, Wk, bk, Wv, bv, Wo, sink,
           **_ignored):
    positions = np.asarray(positions)
    hidden_states = np.asarray(hidden_states, dtype=np.float32)
    Wq = np.asarray(Wq, dtype=np.float32)
    Wk = np.asarray(Wk, dtype=np.float32)
    Wv = np.asarray(Wv, dtype=np.float32)
    Wo = np.asarray(Wo, dtype=np.float32)
    bq = np.asarray(bq, dtype=np.float32)
    bk = np.asarray(bk, dtype=np.float32)
    bv = np.asarray(bv, dtype=np.float32)
    sink = np.asarray(sink, dtype=np.float32)

    # host-derived tables
    half = D // 2
    inv_freq = 1.0 / (THETA ** (np.arange(half, dtype=np.float64) / half))
    ang = positions[0].astype(np.float64)[None, :] * inv_freq[:, None]  # [64,S]
    cos64 = np.cos(ang).astype(np.float32)
    sin64 = np.sin(ang).astype(np.float32)
    cosd = np.ascontiguousarray(np.concatenate([cos64, cos64], axis=0))
    # signed sin: top half -sin (x1c - x2s), bottom half +sin (x2c + x1s)
    sind = np.ascontiguousarray(np.concatenate([-sin64, sin64], axis=0))
    r, cidx = np.arange(128)[:, None], np.arange(128)[None, :]
    m0 = (r <= cidx).astype(np.float32)
    m8 = (r > cidx).astype(np.float32)
    ident = np.eye(128, dtype=np.float32)
    pswap = np.zeros((128, 128), dtype=np.float32)
    pswap[np.arange(128), (np.arange(128) + 64) % 128] = 1.0

    # hsT tiled: [KT*NCH*128, CH]; row block (kt*NCH + c) = hsT tile
    hsT_full = _round_f32r(np.ascontiguousarray(hidden_states[0].T))
    hsT_t = np.ascontiguousarray(
        hsT_full.reshape(KT, 128, NCH, CH).transpose(0, 2, 1, 3)
        .reshape(KT * NCH * 128, CH))
    esink_all = np.exp(sink.astype(np.float64)).astype(np.float32)

    in_maps = []
    for core in range(NCORES):
        qs = slice(core * HQ * D, (core + 1) * HQ * D)
        ks = slice(core * D, (core + 1) * D)
        # weights pre-tiled: [128, KT*cols] with kt blocks side by side
        wq_c = _round_f32r(Wq[:, qs]).reshape(KT, 128, HQ * D)
        wq_t = np.ascontiguousarray(
            wq_c.transpose(1, 0, 2).reshape(128, KT * HQ * D))
        wk_t = np.ascontiguousarray(
            _round_f32r(Wk[:, ks]).reshape(KT, 128, D)
            .transpose(1, 0, 2).reshape(128, KT * D))
        wv_t = np.ascontiguousarray(
            _round_f32r(Wv[:, ks]).reshape(KT, 128, D)
            .transpose(1, 0, 2).reshape(128, KT * D))
        # wo tiled: [HQ*NE*128, CH]
        wo_t = np.ascontiguousarray(
            _round_f32r(Wo[qs, :]).reshape(HQ, 128, NE, CH)
            .transpose(0, 2, 1, 3).reshape(HQ * NE * 128, CH))
        in_maps.append(dict(
            hsT=hsT_t, wq=wq_t, wk=wk_t, wv=wv_t, wo=wo_t,
            bq=np.ascontiguousarray(bq[qs].reshape(HQ, D).T),
            bk=np.ascontiguousarray(bk[ks].reshape(D, 1)),
            bv=np.ascontiguousarray(bv[ks].reshape(D, 1)),
            cosd=cosd, sind=sind, m0=m0, m8=m8,
            esink=np.ascontiguousarray(
                esink_all[core * HQ:(core + 1) * HQ].reshape(1, HQ)),
            ident=ident, pswap=pswap,
            onc=np.ones((128, 1), dtype=np.float32),
            onr=np.ones((1, 128), dtype=np.float32),
        ))

    global _LAST_IN_MAPS
    _LAST_IN_MAPS = in_maps
    nc = _get_nc()
    res = run_bass_kernel_spmd(nc, in_maps, list(range(NCORES)))
    out = np.zeros((S, HID), dtype=np.float64)
    for core in range(NCORES):
        out += res.results[core]["o_part"].astype(np.float64)
    return out.astype(np.float32).reshape(B, S, HID)


# revision 12
# speedup vs baseline: 1.2314x; 1.1837x over previous
"""Self-contained Trainium2 Bass kernel for nn_MiMoMoeAttention.

Tensor-parallel over heads across 8 NeuronCores: each core owns 4 query
heads + 1 kv head (one GQA group); q/k/v projections column-sharded,
o_proj row-sharded, partial outputs summed on the host.

All matmuls run as fp32r (fp32 with mantissa RNE-rounded to 11 bits,
single-pass full-rate on the PE). Host pre-rounds weights/activations to
the fp32r encoding and pre-tiles them for contiguous 256KB DMAs.
"""
import numpy as np
from contextlib import ExitStack

from concourse import bacc
import concourse.tile as tile
import concourse.mybir as mybir
from concourse.alu_op_type import AluOpType
from concourse.bass_utils import run_bass_kernel_spmd

dt = mybir.dt
AF = mybir.ActivationFunctionType

B, S, HID = 1, 2048, 4096
H, HK, D = 32, 8, 128
WIN = 1024
THETA = 1000000.0
NCORES = 8
HQ = H // NCORES            # 4 query heads per core
CH = 512                    # token chunk width
NCH = S // CH               # 4 chunks
KT = HID // 128             # 32 contraction tiles
NE = HID // CH              # 8 o_proj column chunks
CBLK = CH // 128            # 4 query blocks per chunk
WBLK = WIN // 128           # 8 blocks lookback
SCALE = float(D) ** -0.5
MASK_NEG = -30000.0


def _round_f32r(x: np.ndarray) -> np.ndarray:
    """RNE-round fp32 to 11 explicit mantissa bits (the fp32r encoding)."""
    u = np.ascontiguousarray(x, dtype=np.float32).view(np.uint32)
    r = (u.astype(np.uint64) + 0x7FF + ((u >> 12) & 1)) & 0xFFFFF000
    return r.astype(np.uint32).view(np.float32)


def _build():
    nc = bacc.Bacc("TRN2", target_bir_lowering=False, debug=False,
                   num_devices=NCORES)
    f32, f32r = dt.float32, dt.float32r
    # hsT tiled host-side: row block (kt*NCH + c) holds hsT[kt, :, chunk c]
    hsT = nc.dram_tensor("hsT", [KT * NCH * 128, CH], f32r,
                         kind="ExternalInput").ap()
    # weights pre-tiled so SBUF destination is a contiguous 2D copy
    wq = nc.dram_tensor("wq", [128, KT * HQ * D], f32r,
                        kind="ExternalInput").ap()
    wk = nc.dram_tensor("wk", [128, KT * D], f32r, kind="ExternalInput").ap()
    wv = nc.dram_tensor("wv", [128, KT * D], f32r, kind="ExternalInput").ap()
    # wo tiled: row block (jt*NE + e) holds Wo[jt*128:(jt+1)*128, e*CH:+CH]
    wo = nc.dram_tensor("wo", [HQ * NE * 128, CH], f32r,
                        kind="ExternalInput").ap()
    bq = nc.dram_tensor("bq", [128, HQ], f32, kind="ExternalInput").ap()
    bk = nc.dram_tensor("bk", [128, 1], f32, kind="ExternalInput").ap()
    bv = nc.dram_tensor("bv", [128, 1], f32, kind="ExternalInput").ap()
    cosd = nc.dram_tensor("cosd", [128, S], f32, kind="ExternalInput").ap()
    sind = nc.dram_tensor("sind", [128, S], f32, kind="ExternalInput").ap()
    m0 = nc.dram_tensor("m0", [128, 128], f32, kind="ExternalInput").ap()
    m8 = nc.dram_tensor("m8", [128, 128], f32, kind="ExternalInput").ap()
    esink = nc.dram_tensor("esink", [1, HQ], f32, kind="ExternalInput").ap()
    ident = nc.dram_tensor("ident", [128, 128], f32, kind="ExternalInput").ap()
    pswap = nc.dram_tensor("pswap", [128, 128], f32r, kind="ExternalInput").ap()
    onc = nc.dram_tensor("onc", [128, 1], f32r, kind="ExternalInput").ap()
    onr = nc.dram_tensor("onr", [1, 128], f32r, kind="ExternalInput").ap()
    out = nc.dram_tensor("o_part", [S, HID], f32, kind="ExternalOutput").ap()

    with tile.TileContext(nc) as tc, ExitStack() as ctx:
        const = ctx.enter_context(tc.tile_pool(name="const", bufs=1))
        keep = ctx.enter_context(tc.tile_pool(name="keep", bufs=1))
        work = ctx.enter_context(tc.tile_pool(name="work", bufs=1))
        ps = ctx.enter_context(tc.tile_pool(name="ps", bufs=1, space="PSUM"))

        # ---- preload constants / weights -------------------------------
        wq_sb = const.tile([128, KT * HQ * D], f32r, tag="wq", name="wq_sb")
        nc.sync.dma_start(wq_sb[:], wq)
        wk_sb = const.tile([128, KT * D], f32r, tag="wk", name="wk_sb")
        nc.sync.dma_start(wk_sb[:], wk)
        wv_sb = const.tile([128, KT * D], f32r, tag="wv", name="wv_sb")
        nc.sync.dma_start(wv_sb[:], wv)
        bq_sb = const.tile([128, HQ], f32, tag="bq", name="bq_sb")
        nc.sync.dma_start(bq_sb[:], bq)
        bk_sb = const.tile([128, 1], f32, tag="bk", name="bk_sb")
        nc.sync.dma_start(bk_sb[:], bk)
        bv_sb = const.tile([128, 1], f32, tag="bv", name="bv_sb")
        nc.sync.dma_start(bv_sb[:], bv)
        m0_sb = const.tile([128, 128], f32, tag="m0", name="m0_sb")
        nc.sync.dma_start(m0_sb[:], m0)
        m8_sb = const.tile([128, 128], f32, tag="m8", name="m8_sb")
        nc.sync.dma_start(m8_sb[:], m8)
        es_sb = const.tile([1, HQ], f32, tag="es", name="es_sb")
        nc.sync.dma_start(es_sb[:], esink)
        id_sb = const.tile([128, 128], f32, tag="ident", name="id_sb")
        nc.sync.dma_start(id_sb[:], ident)
        pw_sb = const.tile([128, 128], f32r, tag="pswap", name="pw_sb")
        nc.sync.dma_start(pw_sb[:], pswap)
        ones_c = const.tile([128, 1], f32r, tag="ones_c", name="ones_c")
        nc.sync.dma_start(ones_c[:], onc)
        ones_r = const.tile([1, 128], f32r, tag="ones_r", name="ones_r")
        nc.sync.dma_start(ones_r[:], onr)

        # persistent rotated K (d-major) and V (t-major) for all tokens
        krotT = keep.tile([128, S], f32r, tag="krotT", name="krotT")
        v_all = keep.tile([128, S], f32r, tag="v_all", name="v_all")

        def rope(dst, src_sb, swap_ps, cos_sb, sin_sb):
            """dst = src*cos + swap*sinS, full 128-partition ops.
            sinS has rows 0:64 = -sin (so top half gets x1c - x2s) and
            rows 64:128 = +sin (bottom half gets x2c + x1s)."""
            t1 = work.tile([128, CH], f32, tag="r1", bufs=2, name="t1")
            nc.vector.tensor_tensor(t1[:], src_sb[:], cos_sb[:],
                                    op=AluOpType.mult)
            t2 = work.tile([128, CH], f32, tag="r2", bufs=2, name="t2")
            nc.vector.tensor_tensor(t2[:], swap_ps[:], sin_sb[:],
                                    op=AluOpType.mult)
            nc.vector.tensor_tensor(dst, t1[:], t2[:], op=AluOpType.add)

        def attn_pass(heads, qrot, blo, bhi, s0, an_out):
            """Sliding-window attention for a pair of heads, j-outer,
            software-pipelined so PE never waits on the exp."""
            at = {}
            dn = {}
            for idx, h in enumerate(heads):
                at[h] = ps.tile([128, CH], f32, tag=f"a{idx}", name="at_ps")
                nc.vector.memset(at[h][:], 0.0)
                dn[h] = ps.tile([1, CH], f32, tag=f"a{idx + 2}", name="dn_ps")
                nc.vector.memset(dn[h][:], 0.0)
            lgi = 0
            pend = []

            def consume(item):
                h, E, c0, w, j = item
                nc.tensor.matmul(dn[h][:, c0:c0 + w], ones_c[:], E[:, :w],
                                 start=False, stop=True)
                nc.tensor.matmul(at[h][:, c0:c0 + w],
                                 v_all[:, j * 128:(j + 1) * 128],
                                 E[:, :w], start=False, stop=True)

            for j in range(max(0, blo - WBLK), bhi + 1):
                lo, hi = max(j, blo), min(j + WBLK, bhi)
                c0 = (lo - blo) * 128
                w = (hi - lo + 1) * 128
                has_m0 = lo == j
                has_m8 = hi == j + WBLK
                for h in heads:
                    lg = ps.tile([128, CH], f32,
                                 tag=("c0", "c1", "b0", "b1")[lgi % 4],
                                 name="lg")
                    lgi += 1
                    nc.tensor.matmul(lg[:, :w],
                                     krotT[:, j * 128:(j + 1) * 128],
                                     qrot[h][:, c0:c0 + w],
                                     start=True, stop=True)
                    E = work.tile([128, CH], f32r, tag="E", bufs=5, name="E")
                    nc.scalar.activation(E[:, :w], lg[:, :w], AF.Exp,
                                         scale=SCALE)
                    if has_m0:
                        nc.vector.tensor_tensor(E[:, 0:128], E[:, 0:128],
                                                m0_sb[:], op=AluOpType.mult)
                    if has_m8:
                        nc.vector.tensor_tensor(E[:, w - 128:w],
                                                E[:, w - 128:w],
                                                m8_sb[:], op=AluOpType.mult)
                    pend.append((h, E, c0, w, j))
                while len(pend) > 4:
                    consume(pend.pop(0))
            for item in pend:
                consume(item)
            for idx, h in enumerate(heads):
                lnd = work.tile([1, CH], f32, tag="lnd", bufs=2, name="lnd")
                nc.scalar.activation(lnd[:], dn[h][:], AF.Ln,
                                     bias=es_sb[0:1, h:h + 1])
                rcp = work.tile([1, CH], f32r, tag="rcp", bufs=2, name="rcp")
                nc.scalar.activation(rcp[:], lnd[:], AF.Exp, scale=-1.0)
                rb_ps = ps.tile([128, CH], f32, tag=f"b{idx}", name="rb_ps")
                nc.tensor.matmul(rb_ps[:], ones_r[:], rcp[:],
                                 start=True, stop=True)
                rb_sb = work.tile([128, CH], f32, tag="rb", bufs=2,
                                  name="rb_sb")
                nc.scalar.copy(rb_sb[:], rb_ps[:])
                an = work.tile([128, CH], f32r, tag="an", bufs=4, name="an")
                nc.vector.tensor_tensor(an[:], at[h][:], rb_sb[:],
                                        op=AluOpType.mult)
                an_out[h] = an

        def qprep(jt, q_ps, cos_sb, sin_sb):
            qT_sb = work.tile([128, CH], f32r, tag="pt", bufs=3, name="qT_sb")
            nc.scalar.activation(qT_sb[:], q_ps[:], AF.Identity,
                                 bias=bq_sb[:, jt:jt + 1])
            qsw = ps.tile([128, CH], f32, tag=f"a{jt}", name="qsw")
            nc.tensor.matmul(qsw[:], pw_sb[:], qT_sb[:], start=True, stop=True)
            qr = work.tile([128, CH], f32r, tag="qrot", bufs=4, name="qr")
            rope(qr[:], qT_sb[:], qsw[:], cos_sb[:], sin_sb[:])
            return qr

        for c in range(NCH):
            s0 = c * CH
            cos_sb = work.tile([128, CH], f32, tag="cos", bufs=1, name="cos_sb")
            nc.sync.dma_start(cos_sb[:], cosd[:, s0:s0 + CH])
            sin_sb = work.tile([128, CH], f32, tag="sin", bufs=1, name="sin_sb")
            nc.sync.dma_start(sin_sb[:], sind[:, s0:s0 + CH])

            # ---- fused q/k/v projection for this token chunk ----------
            q_ps = [ps.tile([128, CH], f32, tag=f"a{jt}", name=f"q_ps{jt}")
                    for jt in range(HQ)]
            k_ps = ps.tile([128, CH], f32, tag="b0", name="k_ps")
            v_ps = ps.tile([128, CH], f32, tag="b1", name="v_ps")
            for kt in range(KT):
                hst = work.tile([128, CH], f32r, tag="hst", bufs=6, name="hst")
                nc.sync.dma_start(
                    hst[:], hsT[(kt * NCH + c) * 128:(kt * NCH + c + 1) * 128])
                first, last = kt == 0, kt == KT - 1
                for jt in range(HQ):
                    nc.tensor.matmul(
                        q_ps[jt][:],
                        wq_sb[:, kt * HQ * D + jt * D:kt * HQ * D + (jt + 1) * D],
                        hst[:], start=first, stop=last)
                nc.tensor.matmul(k_ps[:], wk_sb[:, kt * D:(kt + 1) * D],
                                 hst[:], start=first, stop=last)
                nc.tensor.matmul(v_ps[:], wv_sb[:, kt * D:(kt + 1) * D],
                                 hst[:], start=first, stop=last)

            # ---- K: bias, half-swap (PE), rope -> krotT[:, chunk] -----
            kT_sb = work.tile([128, CH], f32r, tag="pt", bufs=3, name="kT_sb")
            nc.scalar.activation(kT_sb[:], k_ps[:], AF.Identity, bias=bk_sb[:])
            ksw = ps.tile([128, CH], f32, tag="b0", name="ksw")
            nc.tensor.matmul(ksw[:], pw_sb[:], kT_sb[:], start=True, stop=True)
            rope(krotT[:, s0:s0 + CH], kT_sb[:], ksw[:], cos_sb[:], sin_sb[:])

            # ---- V: bias, transpose to t-major -> v_all[:, chunk] -----
            vT_sb = work.tile([128, CH], f32, tag="pt", bufs=3, name="vT_sb")
            nc.scalar.activation(vT_sb[:], v_ps[:], AF.Identity, bias=bv_sb[:])
            for i in range(CBLK):
                vt = ps.tile([128, 128], f32, tag="b1", name="vt")
                nc.tensor.transpose(vt[:], vT_sb[:, i * 128:(i + 1) * 128],
                                    id_sb[:])
                nc.scalar.copy(
                    v_all[:, s0 + i * 128:s0 + (i + 1) * 128], vt[:])

            # ---- Q prep + attention in two head-pair passes -----------
            blo, bhi = c * CBLK, c * CBLK + CBLK - 1
            qrot = {}
            an_out = {}
            qrot[0] = qprep(0, q_ps[0], cos_sb, sin_sb)
            qrot[1] = qprep(1, q_ps[1], cos_sb, sin_sb)
            attn_pass((0, 1), qrot, blo, bhi, s0, an_out)
            qrot[2] = qprep(2, q_ps[2], cos_sb, sin_sb)
            qrot[3] = qprep(3, q_ps[3], cos_sb, sin_sb)
            attn_pass((2, 3), qrot, blo, bhi, s0, an_out)

            # ---- o_proj (row shard): out[s, e] += attn.T @ Wo ---------
            for e in range(NE):
                wo_t = []
                for jt in range(HQ):
                    wt = work.tile([128, CH], f32r, tag="wo", bufs=8,
                                   name="wo_t")
                    nc.sync.dma_start(
                        wt[:],
                        wo[(jt * NE + e) * 128:(jt * NE + e + 1) * 128])
                    wo_t.append(wt)
                for sb in range(CBLK):
                    o_ps = ps.tile([128, CH], f32, tag=f"a{sb}",
                                   name="o_ps")
                    for jt in range(HQ):
                        nc.tensor.matmul(
                            o_ps[:], an_out[jt][:, sb * 128:(sb + 1) * 128],
                            wo_t[jt][:], start=jt == 0, stop=jt == HQ - 1)
                    o_sb = work.tile([128, CH], f32, tag="osb", bufs=3,
                                     name="o_sb")
                    if (e + sb) % 2 == 0:
                        nc.scalar.copy(o_sb[:], o_ps[:])
                    else:
                        nc.vector.tensor_copy(o_sb[:], o_ps[:])
                    nc.sync.dma_start(
                        out[s0 + sb * 128:s0 + (sb + 1) * 128,
                            e * CH:(e + 1) * CH], o_sb[:])

    nc.compile()
    return nc


_CACHED = None
_LAST_IN_MAPS = None


def _get_nc():
    global _CACHED
    if _CACHED is None:
        _CACHED = _build()
    return _CACHED


def kernel(positions, hidden_states, Wq, bq, Wk, bk, Wv, bv, Wo, sink,
           **_ignored):
    positions = np.asarray(positions)
    hidden_states = np.asarray(hidden_states, dtype=np.float32)
    Wq = np.asarray(Wq, dtype=np.float32)
    Wk = np.asarray(Wk, dtype=np.float32)
    Wv = np.asarray(Wv, dtype=np.float32)
    Wo = np.asarray(Wo, dtype=np.float32)
    bq = np.asarray(bq, dtype=np.float32)
    bk = np.asarray(bk, dtype=np.float32)
    bv = np.asarray(bv, dtype=np.float32)
    sink = np.asarray(sink, dtype=np.float32)

    # host-derived tables
    half = D // 2
    inv_freq = 1.0 / (THETA ** (np.arange(half, dtype=np.float64) / half))
    ang = positions[0].astype(np.float64)[None, :] * inv_freq[:, None]  # [64,S]
    cos64 = np.cos(ang).astype(np.float32)
    sin64 = np.sin(ang).astype(np.float32)
    cosd = np.ascontiguousarray(np.concatenate([cos64, cos64], axis=0))
    # signed sin: top half -sin (x1c - x2s), bottom half +sin (x2c + x1s)
    sind = np.ascontiguousarray(np.concatenate([-sin64, sin64], axis=0))
    r, cidx = np.arange(128)[:, None], np.arange(128)[None, :]
    m0 = (r <= cidx).astype(np.float32)
    m8 = (r > cidx).astype(np.float32)
    ident = np.eye(128, dtype=np.float32)
    pswap = np.zeros((128, 128), dtype=np.float32)
    pswap[np.arange(128), (np.arange(128) + 64) % 128] = 1.0

    # hsT tiled: [KT*NCH*128, CH]; row block (kt*NCH + c) = hsT tile
    hsT_full = _round_f32r(np.ascontiguousarray(hidden_states[0].T))
    hsT_t = np.ascontiguousarray(
        hsT_full.reshape(KT, 128, NCH, CH).transpose(0, 2, 1, 3)
        .reshape(KT * NCH * 128, CH))
    esink_all = np.exp(sink.astype(np.float64)).astype(np.float32)

    in_maps = []
    for core in range(NCORES):
        qs = slice(core * HQ * D, (core + 1) * HQ * D)
        ks = slice(core * D, (core + 1) * D)
        # weights pre-tiled: [128, KT*cols] with kt blocks side by side
        wq_c = _round_f32r(Wq[:, qs]).reshape(KT, 128, HQ * D)
        wq_t = np.ascontiguousarray(
            wq_c.transpose(1, 0, 2).reshape(128, KT * HQ * D))
        wk_t = np.ascontiguousarray(
            _round_f32r(Wk[:, ks]).reshape(KT, 128, D)
            .transpose(1, 0, 2).reshape(128, KT * D))
        wv_t = np.ascontiguousarray(
            _round_f32r(Wv[:, ks]).reshape(KT, 128, D)
            .transpose(1, 0, 2).reshape(128, KT * D))
        # wo tiled: [HQ*NE*128, CH]
        wo_t = np.ascontiguousarray(
            _round_f32r(Wo[qs, :]).reshape(HQ, 128, NE, CH)
            .transpose(0, 2, 1, 3).reshape(HQ * NE * 128, CH))
        in_maps.append(dict(
            hsT=hsT_t, wq=wq_t, wk=wk_t, wv=wv_t, wo=wo_t,
            bq=np.ascontiguousarray(bq[qs].reshape(HQ, D).T),
            bk=np.ascontiguousarray(bk[ks].reshape(D, 1)),
            bv=np.ascontiguousarray(bv[ks].reshape(D, 1)),
            cosd=cosd, sind=sind, m0=m0, m8=m8,
            esink=np.ascontiguousarray(
                esink_all[core * HQ:(core + 1) * HQ].reshape(1, HQ)),
            ident=ident, pswap=pswap,
            onc=np.ones((128, 1), dtype=np.float32),
            onr=np.ones((1, 128), dtype=np.float32),
        ))

    global _LAST_IN_MAPS
    _LAST_IN_MAPS = in_maps
    nc = _get_nc()
    res = run_bass_kernel_spmd(nc, in_maps, list(range(NCORES)))
    out = np.zeros((S, HID), dtype=np.float64)
    for core in range(NCORES):
        out += res.results[core]["o_part"].astype(np.float64)
    return out.astype(np.float32).reshape(B, S, HID)
